# revision 33
# baseline (speedup 1.0000x reference)
"""Trainium2 Bass kernel for nn_AnswerOnlyReward (ragged_sequence).

Strategy:
  - 1024 graphs x 4096 edges, uniform layout. Shard 128 contiguous graphs
    per core across 8 NeuronCores; graphs are independent -> no collectives.
  - On-core layout: one graph per SBUF partition. All per-graph segment
    reductions are fused single-pass ops with accum_out (accumulating ops
    run at 1 elem/lane/cycle, so the work is split across three engines):
      * VectorE: 11 of the 16 (answer x chunk) masked-compare-sum units as
        fused scalar_tensor_tensor(is_equal, mult, accum), plus
        sum(scores*sel).
      * ScalarE: the remaining 5 units via a two-pass activation trick on
        premasked data (u = Square(hm - ans); accum += Relu(1 - u)), plus
        nsel = sum(sel), sum(scores), sum(scores^2) via activation accum.
      * GpSimd: builds the premasked tile hm = min(ht, sel*30000 - 1) that
        ScalarE's compare needs.
  - Inputs are DMA'd in chunks; compute is paced by chunk arrival.
  - The per-graph reduction partials are DMA'd out; the tiny O(G) scalar
    epilogue (reward/precision/recall/f1) runs on the host during
    unsharding.
"""

import numpy as np

from concourse import bass, mybir
from concourse.bass_utils import run_bass_kernel_spmd

G = 1024
EPG = 4096
NCORES = 8
GPC = G // NCORES          # 128 graphs per core = 128 partitions
APG = 4                    # answers per graph (uniform)

AF = mybir.ActivationFunctionType
OP = mybir.AluOpType
DT = mybir.dt

SUCCESS_REWARD = 1.0
FAILURE_REWARD = 1e-8
BETA_REACH = 0.1
BETA_SCORE = 0.5

NCH = 4                    # chunks over the 2*EPG ht axis
HCH = (2 * EPG) // NCH     # 2048 columns per ht chunk
SCH = 2                    # chunks over the EPG scores axis
SCW = EPG // SCH           # 2048 columns per scores chunk

# (answer, chunk) units computed on ScalarE (on Pool-premasked data).
ACT_CHUNKS = (0, 1)
ACT_UNITS = set()

# out_t columns:
# 0        nsel
# 1..2     sumsm partials (SCH)
# 3..4     sums partials (SCH)
# 5..6     sumsq partials (SCH)
# 8..23    hitsum partials -> 8 + c*APG + a
OUTW = 24


def _build():
    nc = bass.Bass()

    ht_e = nc.declare_dram_parameter("ht", [GPC, 2 * EPG], DT.int32, isOutput=False)
    scores_e = nc.declare_dram_parameter("scores", [GPC, EPG], DT.float32, isOutput=False)
    sel2_e = nc.declare_dram_parameter("sel2", [GPC, 2 * EPG], DT.uint8, isOutput=False)
    meta_e = nc.declare_dram_parameter("meta", [GPC, 8], DT.float32, isOutput=False)
    out_e = nc.declare_dram_parameter("out", [GPC, OUTW], DT.float32, isOutput=True)

    with (
        nc.Block() as block,
        nc.semaphore("dma_sem") as dma,
        nc.semaphore("dma_a_sem") as dma_a,
        nc.semaphore("v_sem") as v_sem,
        nc.semaphore("a_sem") as a_sem,
        nc.sbuf_tensor("ht_t", [GPC, 2 * EPG], DT.int32) as ht,
        nc.sbuf_tensor("s_t", [GPC, EPG], DT.float32) as s,
        nc.sbuf_tensor("m8_t", [GPC, 2 * EPG], DT.uint8) as m8,
        nc.sbuf_tensor("junk_eq", [GPC, 4096], DT.bfloat16) as junk_eq,
        nc.sbuf_tensor("junk_eq2", [GPC, 4096], DT.bfloat16) as junk_eq2,
        nc.sbuf_tensor("junk_sm", [GPC, SCW], DT.float32) as junk_sm,
        nc.sbuf_tensor("junk_act", [GPC, HCH], DT.bfloat16) as junk_act,
        nc.sbuf_tensor("junk_sp", [GPC, 512], DT.float32) as junk_sp,
        nc.sbuf_tensor("meta_t", [GPC, 8], DT.float32) as meta,
        nc.sbuf_tensor("out_t", [GPC, OUTW], DT.float32) as out_t,
    ):
        # sync queue (dma): ht chunks (1024, 3072, 4096 cols) | out
        # scalar queue (dma_a): meta | m8 c0 (1024) | m8 rest | s c0 | s c1
        HT_CH = [(0, 4096), (4096, 8192)]
        TH_HT = [16, 32]
        TH_META = 16
        TH_M8 = 32
        TH_S = 48

        @block.sync
        def _(sync):
            for (c0, c1) in HT_CH:
                sync.dma_start(out=ht[:, c0:c1],
                               in_=ht_e[:, c0:c1]).then_inc(dma, 16)
            sync.wait_ge(v_sem, 1)
            sync.wait_ge(a_sem, 4)
            sync.dma_start(out=out_e[:, :], in_=out_t[:, :]).then_inc(dma, 16)
            sync.wait_ge(dma, 48)

        @block.scalar
        def _(sc):
            sc.dma_start(out=meta[:, :], in_=meta_e[:, :]).then_inc(dma_a, 16)
            sc.dma_start(out=m8[:, :], in_=sel2_e[:, :]).then_inc(dma_a, 16)
            sc.dma_start(out=s[:, :], in_=scores_e[:, :]).then_inc(dma_a, 16)
            # nsel = sum(sel) over first half of m8 (two chunk partials)
            sc.wait_ge(dma_a, TH_M8)  # m8 landed
            sc.activation(junk_act[:, :], m8[:, 0:HCH], AF.Copy,
                          accum_out=out_t[:, 0:1])
            sc.activation(junk_act[:, :], m8[:, HCH:EPG], AF.Copy,
                          accum_out=out_t[:, 7:8]).then_inc(a_sem, 1)
            # sums / sumsq partials
            sc.wait_ge(dma_a, TH_S)
            for c in range(SCH):
                sl = s[:, c * SCW:(c + 1) * SCW]
                sc.activation(junk_act[:, :], sl, AF.Copy,
                              accum_out=out_t[:, 3 + c:4 + c])
                sc.activation(junk_act[:, :], sl, AF.Square,
                              accum_out=out_t[:, 5 + c:6 + c]).then_inc(a_sem, 1)
            # spacers so accumulator read-outs land before the final inc
            sc.activation(junk_act[:, 0:512], m8[:, 0:512], AF.Copy)
            sc.activation(junk_act[:, 0:512], m8[:, 0:512], AF.Copy)
            sc.activation(junk_act[:, 0:512], m8[:, 0:512],
                          AF.Copy).then_inc(a_sem, 1)
            # a_sem total: 1 (nsel) + 5 (units) + 2 (scores) + 2 (spacer+final)

        @block.vector
        def _(v):
            v.wait_ge(dma_a, TH_M8)   # meta + m8
            # VectorE hit units: fused masked compare+sum per (chunk, answer)
            for c, (c0, c1) in enumerate(HT_CH):
                v.wait_ge(dma, TH_HT[c])
                cs = slice(c0, c1)
                jk = junk_eq if c % 2 == 0 else junk_eq2
                for a in range(APG):
                    col = 8 + c * APG + a
                    v.scalar_tensor_tensor(
                        out=jk[:, 0:c1 - c0], in0=ht[:, cs],
                        scalar=meta[:, a:a + 1],
                        in1=m8[:, cs], op0=OP.is_equal, op1=OP.mult,
                        accum_out=out_t[:, col:col + 1])
                if c == 0:
                    # sumsm partials between the two ht chunk blocks
                    v.wait_ge(dma_a, TH_S)
                    for sc_ in range(SCH):
                        v.scalar_tensor_tensor(
                            out=junk_sm[:, :],
                            in0=s[:, sc_ * SCW:(sc_ + 1) * SCW],
                            scalar=1.0, in1=m8[:, sc_ * SCW:(sc_ + 1) * SCW],
                            op0=OP.mult, op1=OP.mult,
                            accum_out=out_t[:, 1 + sc_:2 + sc_])
            # spacers so the last accumulator read-out lands before the
            # output DMA is released
            v.tensor_scalar(junk_sp[:, :], junk_sm[:, 0:512], 1.0, None, OP.mult)
            v.tensor_scalar(junk_sp[:, :], junk_sm[:, 0:512], 1.0, None, OP.mult)
            v.tensor_scalar(junk_sp[:, :], junk_sm[:, 0:512], 1.0, None,
                            OP.mult).then_inc(v_sem, 1)

    return nc


_NC_CACHE = None


def _get_nc():
    global _NC_CACHE
    if _NC_CACHE is None:
        _NC_CACHE = _build()
    return _NC_CACHE


def _run(in_maps, trace=False):
    nc = _get_nc()
    return run_bass_kernel_spmd(nc, in_maps, core_ids=list(range(NCORES)),
                                trace=trace)


def _make_in_maps(inputs):
    heads = np.asarray(inputs["edge_heads"], dtype=np.int32).reshape(NCORES, GPC, EPG)
    tails = np.asarray(inputs["edge_tails"], dtype=np.int32).reshape(NCORES, GPC, EPG)
    ht = np.concatenate([heads, tails], axis=2)                 # [8, 128, 8192]
    scores = np.ascontiguousarray(
        np.asarray(inputs["edge_scores"], dtype=np.float32).reshape(NCORES, GPC, EPG))
    sel = np.asarray(inputs["selected_mask"]).astype(np.uint8).reshape(NCORES, GPC, EPG)
    sel2 = np.concatenate([sel, sel], axis=2)                   # [8, 128, 8192]

    aptr = np.asarray(inputs["answer_ptr"]).astype(np.int64)
    aeid = np.asarray(inputs["answer_entity_ids"])
    counts = (aptr[1:] - aptr[:-1]).astype(np.float32)          # [G]
    apg = aeid.shape[0] // G
    ans2d = aeid.reshape(G, apg).astype(np.float32)
    valid = np.arange(apg)[None, :] < counts[:, None]
    anspad = np.where(valid, ans2d, -2.0).astype(np.float32)    # [G, apg]

    meta = np.zeros((G, 8), dtype=np.float32)
    meta[:, 0:APG] = anspad[:, 0:APG]       # VectorE is_equal scalars
    meta[:, 4:4 + APG] = -(anspad[:, 0:APG] + 1.0)  # ScalarE bias = -(ans+1)

    in_maps = []
    for c in range(NCORES):
        g0, g1 = c * GPC, (c + 1) * GPC
        in_maps.append({
            "ht": np.ascontiguousarray(ht[c]),
            "scores": scores[c],
            "sel2": np.ascontiguousarray(sel2[c]),
            "meta": np.ascontiguousarray(meta[g0:g1]),
        })
    return in_maps


def _assemble(results, inputs):
    ocat = np.concatenate([np.asarray(results[c]["out"]) for c in range(NCORES)],
                          axis=0).astype(np.float64)             # [1024, OUTW]
    nsel = ocat[:, 0] + ocat[:, 7]
    sumsm = ocat[:, 1] + ocat[:, 2]
    sums = ocat[:, 3] + ocat[:, 4]
    sumsq = ocat[:, 5] + ocat[:, 6]
    hitsums = ocat[:, 8:8 + 2 * APG].reshape(G, 2, APG).sum(axis=1)

    aptr = np.asarray(inputs["answer_ptr"]).astype(np.int64)
    counts = (aptr[1:] - aptr[:-1]).astype(np.float64)
    succ = np.asarray(inputs["reach_success"]).astype(np.float64)
    rf = np.asarray(inputs["reach_fraction"]).astype(np.float64)

    hits = (hitsums > 0).sum(axis=1).astype(np.float64)

    selcnt = np.maximum(nsel, 1.0)
    p_hits = np.minimum(hits, nsel)
    r_hits = np.minimum(hits, counts)
    precision = np.where(nsel > 0, p_hits / selcnt, 0.0)
    recall = np.where(counts > 0, r_hits / np.maximum(counts, 1.0), 0.0)
    psum = precision + recall
    f1 = np.where(psum > 0, 2 * precision * recall / np.maximum(psum, 1e-12), 0.0)

    mean = sums / EPG
    var = np.maximum(sumsq / EPG - mean * mean, 0.0)
    std = np.maximum(np.sqrt(var), 1e-6)
    score_mean = np.clip((sumsm - nsel * mean) / std / selcnt, -4.0, 4.0)
    reward = (FAILURE_REWARD + succ * (SUCCESS_REWARD - FAILURE_REWARD))
    reward = reward * np.exp(BETA_REACH * rf + BETA_SCORE * score_mean)
    reward = np.maximum(reward, 1e-8)

    pe = np.asarray(inputs["path_exists"]).astype(np.float32)
    rff = rf.astype(np.float32)

    out = np.zeros((21, G), dtype=np.float32)
    out[0] = reward
    out[1] = recall
    out[2] = succ.astype(np.float32)
    out[4] = (nsel == 0).astype(np.float32)
    out[8] = precision
    out[9] = recall
    out[10] = f1
    out[14] = pe
    out[16] = rff
    out[17] = pe
    out[18] = rff
    out[19] = 1.0
    out[20] = 1.0
    return out


def kernel(**inputs) -> np.ndarray:
    in_maps = _make_in_maps(inputs)
    res = _run(in_maps, trace=False)
    return _assemble(res.results, inputs)


def _ensure_ntff_hook():
    """The agent image's antenv lacks axon_hooks; shim it so trace=True
    can register the ctypes NTFF profiling hook."""
    import sys
    import types
    try:
        from antenv import axon_hooks  # noqa: F401
        return
    except ImportError:
        pass
    import antenv
    mod = types.ModuleType("antenv.axon_hooks")
    mod._hook = None

    def set_axon_ntff_profile_hook(h):
        mod._hook = h

    def get_axon_ntff_profile_hook():
        return mod._hook

    mod.set_axon_ntff_profile_hook = set_axon_ntff_profile_hook
    mod.get_axon_ntff_profile_hook = get_axon_ntff_profile_hook
    sys.modules["antenv.axon_hooks"] = mod
    antenv.axon_hooks = mod
    try:
        from trn_agent_boot.trn_boot import _ntff_profile_via_ctypes
        mod._hook = _ntff_profile_via_ctypes("/opt/axon/libaxon_pjrt.so")
    except Exception:
        pass


def kernel_traced(**inputs):
    """Like kernel() but returns (output, exec_time_ns, results_obj)."""
    _ensure_ntff_hook()
    in_maps = _make_in_maps(inputs)
    res = _run(in_maps, trace=True)
    return _assemble(res.results, inputs), res.exec_time_ns, res


# revision 34
# speedup vs baseline: 1.1927x; 1.1927x over previous
"""Trainium2 Bass kernel for nn_AnswerOnlyReward (ragged_sequence).

Strategy:
  - 1024 graphs x 4096 edges, uniform layout. Shard 128 contiguous graphs
    per core across 8 NeuronCores; graphs are independent -> no collectives.
  - On-core layout: one graph per SBUF partition. All per-graph segment
    reductions are fused single-pass ops with accum_out (accumulating ops
    run at 1 elem/lane/cycle, so the work is split across three engines):
      * VectorE: 11 of the 16 (answer x chunk) masked-compare-sum units as
        fused scalar_tensor_tensor(is_equal, mult, accum), plus
        sum(scores*sel).
      * ScalarE: the remaining 5 units via a two-pass activation trick on
        premasked data (u = Square(hm - ans); accum += Relu(1 - u)), plus
        nsel = sum(sel), sum(scores), sum(scores^2) via activation accum.
      * GpSimd: builds the premasked tile hm = min(ht, sel*30000 - 1) that
        ScalarE's compare needs.
  - Inputs are DMA'd in chunks; compute is paced by chunk arrival.
  - The per-graph reduction partials are DMA'd out; the tiny O(G) scalar
    epilogue (reward/precision/recall/f1) runs on the host during
    unsharding.
"""

import numpy as np

from concourse import bass, mybir
from concourse.bass_utils import run_bass_kernel_spmd

G = 1024
EPG = 4096
NCORES = 8
GPC = G // NCORES          # 128 graphs per core = 128 partitions
APG = 4                    # answers per graph (uniform)

AF = mybir.ActivationFunctionType
OP = mybir.AluOpType
DT = mybir.dt

SUCCESS_REWARD = 1.0
FAILURE_REWARD = 1e-8
BETA_REACH = 0.1
BETA_SCORE = 0.5

NCH = 4                    # chunks over the 2*EPG ht axis
HCH = (2 * EPG) // NCH     # 2048 columns per ht chunk
SCH = 2                    # chunks over the EPG scores axis
SCW = EPG // SCH           # 2048 columns per scores chunk

# (answer, chunk) units computed on ScalarE (on Pool-premasked data).
ACT_CHUNKS = (0, 1)
ACT_UNITS = set()

# out_t columns:
# 0        nsel
# 1..2     sumsm partials (SCH)
# 3..4     sums partials (SCH)
# 5..6     sumsq partials (SCH)
# 8..23    hitsum partials -> 8 + c*APG + a
OUTW = 24


def _build():
    nc = bass.Bass()

    ht_e = nc.declare_dram_parameter("ht", [GPC, 2 * EPG], DT.int32, isOutput=False)
    scores_e = nc.declare_dram_parameter("scores", [GPC, EPG], DT.float32, isOutput=False)
    sel2_e = nc.declare_dram_parameter("sel2", [GPC, 2 * EPG], DT.uint8, isOutput=False)
    meta_e = nc.declare_dram_parameter("meta", [GPC, 8], DT.float32, isOutput=False)
    out_e = nc.declare_dram_parameter("out", [GPC, OUTW], DT.float32, isOutput=True)

    with (
        nc.Block() as block,
        nc.semaphore("dma_sem") as dma,
        nc.semaphore("dma_a_sem") as dma_a,
        nc.semaphore("v_sem") as v_sem,
        nc.semaphore("a_sem") as a_sem,
        nc.sbuf_tensor("ht_t", [GPC, 2 * EPG], DT.int32) as ht,
        nc.sbuf_tensor("s_t", [GPC, EPG], DT.float32) as s,
        nc.sbuf_tensor("m8_t", [GPC, 2 * EPG], DT.uint8) as m8,
        nc.sbuf_tensor("junk_eq", [GPC, 4096], DT.bfloat16) as junk_eq,
        nc.sbuf_tensor("junk_eq2", [GPC, 4096], DT.bfloat16) as junk_eq2,
        nc.sbuf_tensor("junk_sm", [GPC, SCW], DT.float32) as junk_sm,
        nc.sbuf_tensor("junk_act", [GPC, HCH], DT.bfloat16) as junk_act,
        nc.sbuf_tensor("junk_sp", [GPC, 512], DT.float32) as junk_sp,
        nc.sbuf_tensor("meta_t", [GPC, 8], DT.float32) as meta,
        nc.sbuf_tensor("out_t", [GPC, OUTW], DT.float32) as out_t,
    ):
        # sync queue (dma): ht chunks (1024, 3072, 4096 cols) | out
        # scalar queue (dma_a): meta | m8 c0 (1024) | m8 rest | s c0 | s c1
        HT_CH = [(0, 2048), (2048, 4096), (4096, 6144), (6144, 8192)]
        TH_HT = [16, 32, 48, 64]
        TH_META = 16
        TH_M8C0 = 32
        TH_M8 = 48
        TH_S = 64

        @block.sync
        def _(sync):
            for (c0, c1) in HT_CH:
                sync.dma_start(out=ht[:, c0:c1],
                               in_=ht_e[:, c0:c1]).then_inc(dma, 16)
            sync.wait_ge(v_sem, 1)
            sync.wait_ge(a_sem, 4)
            sync.dma_start(out=out_e[:, :], in_=out_t[:, :]).then_inc(dma, 16)
            sync.wait_ge(dma, 80)

        @block.scalar
        def _(sc):
            sc.dma_start(out=meta[:, :], in_=meta_e[:, :]).then_inc(dma_a, 16)
            sc.dma_start(out=m8[:, 0:2048], in_=sel2_e[:, 0:2048]
                         ).then_inc(dma_a, 16)
            sc.dma_start(out=m8[:, 2048:2 * EPG], in_=sel2_e[:, 2048:2 * EPG]
                         ).then_inc(dma_a, 16)
            sc.dma_start(out=s[:, :], in_=scores_e[:, :]).then_inc(dma_a, 16)
            # nsel = sum(sel) over first half of m8 (two chunk partials)
            sc.wait_ge(dma_a, TH_M8)  # m8 landed
            sc.activation(junk_act[:, :], m8[:, 0:HCH], AF.Copy,
                          accum_out=out_t[:, 0:1])
            sc.activation(junk_act[:, :], m8[:, HCH:EPG], AF.Copy,
                          accum_out=out_t[:, 7:8]).then_inc(a_sem, 1)
            # sums / sumsq partials
            sc.wait_ge(dma_a, TH_S)
            for c in range(SCH):
                sl = s[:, c * SCW:(c + 1) * SCW]
                sc.activation(junk_act[:, :], sl, AF.Copy,
                              accum_out=out_t[:, 3 + c:4 + c])
                sc.activation(junk_act[:, :], sl, AF.Square,
                              accum_out=out_t[:, 5 + c:6 + c]).then_inc(a_sem, 1)
            # spacers so accumulator read-outs land before the final inc
            sc.activation(junk_act[:, 0:512], m8[:, 0:512], AF.Copy)
            sc.activation(junk_act[:, 0:512], m8[:, 0:512], AF.Copy)
            sc.activation(junk_act[:, 0:512], m8[:, 0:512],
                          AF.Copy).then_inc(a_sem, 1)
            # a_sem total: 1 (nsel) + 5 (units) + 2 (scores) + 2 (spacer+final)

        @block.vector
        def _(v):
            v.wait_ge(dma_a, TH_M8C0)   # meta + first m8 chunk
            # VectorE hit units: fused masked compare+sum per (chunk, answer)
            for c, (c0, c1) in enumerate(HT_CH):
                v.wait_ge(dma, TH_HT[c])
                cs = slice(c0, c1)
                jk = junk_eq if c % 2 == 0 else junk_eq2
                if c == 1:
                    v.wait_ge(dma_a, TH_M8)  # rest of m8
                for a in range(APG):
                    col = 8 + c * APG + a
                    v.scalar_tensor_tensor(
                        out=jk[:, 0:c1 - c0], in0=ht[:, cs],
                        scalar=meta[:, a:a + 1],
                        in1=m8[:, cs], op0=OP.is_equal, op1=OP.mult,
                        accum_out=out_t[:, col:col + 1])
                if c == 2:
                    # sumsm partials interleaved with the hit units
                    v.wait_ge(dma_a, TH_S)
                    for sc_ in range(SCH):
                        v.scalar_tensor_tensor(
                            out=junk_sm[:, :],
                            in0=s[:, sc_ * SCW:(sc_ + 1) * SCW],
                            scalar=1.0, in1=m8[:, sc_ * SCW:(sc_ + 1) * SCW],
                            op0=OP.mult, op1=OP.mult,
                            accum_out=out_t[:, 1 + sc_:2 + sc_])
            # spacers so the last accumulator read-out lands before the
            # output DMA is released
            v.tensor_scalar(junk_sp[:, :], junk_sm[:, 0:512], 1.0, None, OP.mult)
            v.tensor_scalar(junk_sp[:, :], junk_sm[:, 0:512], 1.0, None, OP.mult)
            v.tensor_scalar(junk_sp[:, :], junk_sm[:, 0:512], 1.0, None,
                            OP.mult).then_inc(v_sem, 1)

    return nc


_NC_CACHE = None


def _get_nc():
    global _NC_CACHE
    if _NC_CACHE is None:
        _NC_CACHE = _build()
    return _NC_CACHE


def _run(in_maps, trace=False):
    nc = _get_nc()
    return run_bass_kernel_spmd(nc, in_maps, core_ids=list(range(NCORES)),
                                trace=trace)


def _make_in_maps(inputs):
    heads = np.asarray(inputs["edge_heads"], dtype=np.int32).reshape(NCORES, GPC, EPG)
    tails = np.asarray(inputs["edge_tails"], dtype=np.int32).reshape(NCORES, GPC, EPG)
    ht = np.concatenate([heads, tails], axis=2)                 # [8, 128, 8192]
    scores = np.ascontiguousarray(
        np.asarray(inputs["edge_scores"], dtype=np.float32).reshape(NCORES, GPC, EPG))
    sel = np.asarray(inputs["selected_mask"]).astype(np.uint8).reshape(NCORES, GPC, EPG)
    sel2 = np.concatenate([sel, sel], axis=2)                   # [8, 128, 8192]

    aptr = np.asarray(inputs["answer_ptr"]).astype(np.int64)
    aeid = np.asarray(inputs["answer_entity_ids"])
    counts = (aptr[1:] - aptr[:-1]).astype(np.float32)          # [G]
    apg = aeid.shape[0] // G
    ans2d = aeid.reshape(G, apg).astype(np.float32)
    valid = np.arange(apg)[None, :] < counts[:, None]
    anspad = np.where(valid, ans2d, -2.0).astype(np.float32)    # [G, apg]

    meta = np.zeros((G, 8), dtype=np.float32)
    meta[:, 0:APG] = anspad[:, 0:APG]       # VectorE is_equal scalars
    meta[:, 4:4 + APG] = -(anspad[:, 0:APG] + 1.0)  # ScalarE bias = -(ans+1)

    in_maps = []
    for c in range(NCORES):
        g0, g1 = c * GPC, (c + 1) * GPC
        in_maps.append({
            "ht": np.ascontiguousarray(ht[c]),
            "scores": scores[c],
            "sel2": np.ascontiguousarray(sel2[c]),
            "meta": np.ascontiguousarray(meta[g0:g1]),
        })
    return in_maps


def _assemble(results, inputs):
    ocat = np.concatenate([np.asarray(results[c]["out"]) for c in range(NCORES)],
                          axis=0).astype(np.float64)             # [1024, OUTW]
    nsel = ocat[:, 0] + ocat[:, 7]
    sumsm = ocat[:, 1] + ocat[:, 2]
    sums = ocat[:, 3] + ocat[:, 4]
    sumsq = ocat[:, 5] + ocat[:, 6]
    hitsums = ocat[:, 8:8 + 4 * APG].reshape(G, 4, APG).sum(axis=1)

    aptr = np.asarray(inputs["answer_ptr"]).astype(np.int64)
    counts = (aptr[1:] - aptr[:-1]).astype(np.float64)
    succ = np.asarray(inputs["reach_success"]).astype(np.float64)
    rf = np.asarray(inputs["reach_fraction"]).astype(np.float64)

    hits = (hitsums > 0).sum(axis=1).astype(np.float64)

    selcnt = np.maximum(nsel, 1.0)
    p_hits = np.minimum(hits, nsel)
    r_hits = np.minimum(hits, counts)
    precision = np.where(nsel > 0, p_hits / selcnt, 0.0)
    recall = np.where(counts > 0, r_hits / np.maximum(counts, 1.0), 0.0)
    psum = precision + recall
    f1 = np.where(psum > 0, 2 * precision * recall / np.maximum(psum, 1e-12), 0.0)

    mean = sums / EPG
    var = np.maximum(sumsq / EPG - mean * mean, 0.0)
    std = np.maximum(np.sqrt(var), 1e-6)
    score_mean = np.clip((sumsm - nsel * mean) / std / selcnt, -4.0, 4.0)
    reward = (FAILURE_REWARD + succ * (SUCCESS_REWARD - FAILURE_REWARD))
    reward = reward * np.exp(BETA_REACH * rf + BETA_SCORE * score_mean)
    reward = np.maximum(reward, 1e-8)

    pe = np.asarray(inputs["path_exists"]).astype(np.float32)
    rff = rf.astype(np.float32)

    out = np.zeros((21, G), dtype=np.float32)
    out[0] = reward
    out[1] = recall
    out[2] = succ.astype(np.float32)
    out[4] = (nsel == 0).astype(np.float32)
    out[8] = precision
    out[9] = recall
    out[10] = f1
    out[14] = pe
    out[16] = rff
    out[17] = pe
    out[18] = rff
    out[19] = 1.0
    out[20] = 1.0
    return out


def kernel(**inputs) -> np.ndarray:
    in_maps = _make_in_maps(inputs)
    res = _run(in_maps, trace=False)
    return _assemble(res.results, inputs)


def _ensure_ntff_hook():
    """The agent image's antenv lacks axon_hooks; shim it so trace=True
    can register the ctypes NTFF profiling hook."""
    import sys
    import types
    try:
        from antenv import axon_hooks  # noqa: F401
        return
    except ImportError:
        pass
    import antenv
    mod = types.ModuleType("antenv.axon_hooks")
    mod._hook = None

    def set_axon_ntff_profile_hook(h):
        mod._hook = h

    def get_axon_ntff_profile_hook():
        return mod._hook

    mod.set_axon_ntff_profile_hook = set_axon_ntff_profile_hook
    mod.get_axon_ntff_profile_hook = get_axon_ntff_profile_hook
    sys.modules["antenv.axon_hooks"] = mod
    antenv.axon_hooks = mod
    try:
        from trn_agent_boot.trn_boot import _ntff_profile_via_ctypes
        mod._hook = _ntff_profile_via_ctypes("/opt/axon/libaxon_pjrt.so")
    except Exception:
        pass


def kernel_traced(**inputs):
    """Like kernel() but returns (output, exec_time_ns, results_obj)."""
    _ensure_ntff_hook()
    in_maps = _make_in_maps(inputs)
    res = _run(in_maps, trace=True)
    return _assemble(res.results, inputs), res.exec_time_ns, res
